# revision 27
# baseline (speedup 1.0000x reference)
"""Trainium2 Bass kernel for ConvMessageAggregator.

Computes, for each node n (messages: [N, 16, 688] fp32):
  f1[i] = relu(w10*x[i] + w11*x[i+2] + b1)      i in 0..13   (dilated 2-tap conv)
  f2[i] = relu(w20*f1[i] + w21*f1[i+2] + b2)    i in 0..11
  out   = relu(sum_k mlp_w[k] * f2[6+k] + mlp_b)             -> [N, 688]

Only f2 rows 6..11 are consumed -> f1 rows 6..13 -> x rows 6..15, so the
kernel reads just the last 10 (contiguous) rows of each node (10/16 of the
input bytes).  Sharding: pure data parallel, node axis split across 8 cores;
all scalar params are baked into the instruction stream at trace time.

Per-core pipeline (2048 nodes = 16 tiles of 128 nodes on partitions).
Inputs stay fp32 through the DMA (a cast-on-DMA load measured 1.56x slower
per SDMA engine than a plain copy); conv1's STT runs at 1x rate regardless
of dtype, so it reads fp32 and WRITES fp16 -- everything downstream runs in
fp16 where DVE gets its 2x/4x perf modes:
  DMA  x[128, 10*688] fp32       (SWDGE load, 3.5MB, flat AP)
  DVE  u1 = (x_ot*r1 + x_pv)     STT fp32->fp16  [128, 8*688]
  ACT  f1 = Relu(p1*u1 + b1)     in place, fp16
  DVE  ys = u1_ot*r2             tensor_scalar, 4x mode
  DVE  u2 = ys + u1_pv           tensor_tensor, 2x mode
  ACT  6x G[k] = Relu(s_k*p2*u2[k] + s_k*b2)  in place (relu homogeneity
       folds |mlp_w[k]/w_anchor| into each row's relu)
  DVE/Pool  +- pairwise tree over the 6 G rows (TT add/sub, 2x mode;
       one level-1 pair runs on the otherwise idle GPSIMD engine)
  ACT  out = Relu(tau*w_anchor*t + mlp_b) -> fp32
  DMA  out tile -> DRAM          (HWDGE on sync, separate ring)

vs the first rewrite (258us): no cast-DMA penalty, f2's big ACT replaced by
the 6 fused G ops, the 5-op STT fold (1x only -- STT has no 2x uop) replaced
by a 2x-mode TT tree.  Measured engine busy per core: DVE ~194us (STT 94 +
TT 80 + TS 20), ACT ~174us, DMA ~187us, Pool ~66us -> ~230-245us total.

Tuning notes from the trace loop (things that made it WORSE, kept for the
record): cast-on-DMA loads run the SDMA engines at 14.4 vs 23.1 GB/s/eng;
a 2nd GPSIMD op per tile degrades DVE 2-port-mode ops via the shared SBUF
port (TS 1228->1789ns); TT with out aliasing in0 drops 2x->1x mode;
partition-splitting an op does not shorten it (DVE/ACT time scales with
free-dim size only); stores issued from nc.scalar put their dispatch on the
busy ACT sequencer.  Most important: every run with xin bufs=3 showed a
uniform ~20% slowdown of ALL engines' op durations (independent of total
SBUF footprint -- 161KB configs were slow, this 169KB one is fast); with
shallow prefetch the load DMA's SBUF writes overlap the compute window more
tightly and degrade every engine's SBUF access.  Keep xin at bufs=4.
"""

import sys

for _p in ("/opt/trn_rl_repo",):
    if _p not in sys.path:
        sys.path.insert(0, _p)

import numpy as np

import concourse.bass as bass
import concourse.tile as tile
from concourse import mybir
from concourse.bass_utils import run_bass_kernel_spmd

N_FULL, L, MSG = 16384, 16, 688
N_CORES = 8
N_LOCAL = N_FULL // N_CORES  # 2048
P = 128                      # nodes per tile (partition dim)
NTILES = N_LOCAL // P        # 16
R0, NROWS = 6, 10            # input rows actually used: 6..15 (contiguous)

F32 = mybir.dt.float32
F16 = mybir.dt.float16
AF = mybir.ActivationFunctionType
OP = mybir.AluOpType


def _split_multi_waits(nc):
    """TPB instructions encode at most ONE semaphore wait; this walrus build's
    codegen rejects instructions with more. Hoist extra waits into standalone
    EventSemaphore ops on the same (in-order) sequencer -- semantically
    identical to the attached wait."""
    for func in nc.m.functions:
        for bb in func.blocks:
            insts = list(bb.instructions)
            if not any(
                i.sync_info is not None and len(i.sync_info.on_wait) > 1
                for i in insts
            ):
                continue
            new = []
            for inst in insts:
                si = inst.sync_info
                if si is not None and len(si.on_wait) > 1:
                    waits = list(si.on_wait)
                    for j, w in enumerate(waits[:-1]):
                        new.append(
                            mybir.InstEventSemaphore(
                                name=f"{inst.name}-hoistw{j}",
                                engine=inst.engine,
                                sync_info=mybir.SyncInfo(on_wait=[w], on_update=[]),
                            )
                        )
                    inst.sync_info = mybir.SyncInfo(
                        on_wait=[waits[-1]], on_update=list(si.on_update)
                    )
                new.append(inst)
            bb.instructions = new


def _conv_split(wa, wb):
    """Factor pre[i] = wa*in[i] + wb*in[i+2] as pivot*(in[pv] + r*in[ot]).

    Returns (pivot_weight, ratio, pivot_row_off, other_row_off) with |ratio|<=1.
    """
    if abs(wa) >= abs(wb):
        return wa, (wb / wa if wa != 0.0 else 0.0), 0, 2
    return wb, wa / wb, 2, 0


def build_program(w10, w11, b1, w20, w21, b2, mlp_w, mlp_b):
    nc = bass.Bass(trn_type="TRN2", name="conv_msg_agg")
    x = nc.dram_tensor("x", [N_LOCAL, L, MSG], F32, kind="ExternalInput")
    out = nc.dram_tensor("out", [N_LOCAL, MSG], F32, kind="ExternalOutput")

    p1, r1, pv1, ot1 = _conv_split(w10, w11)
    p2, r2, pv2, ot2 = _conv_split(w20, w21)

    # mlp plan: anchor = argmax |w|; G[k] = s_k*relu(p2*u2[k] + b2) with
    # s_k = |w_k/w_anchor| <= 1 folded into the ACT op (relu homogeneity,
    # s_k > 0).  Sum = w_anchor * sum_k tau_k G[k] via a scalar-free +-
    # pairwise tree.
    nzk = [k for k in range(6) if mlp_w[k] != 0.0]
    anchor = max(nzk, key=lambda k: abs(mlp_w[k])) if nzk else -1
    wa = mlp_w[anchor] if nzk else 0.0

    with tile.TileContext(nc) as tc:
        with (
            tc.tile_pool(name="bias", bufs=1) as pool_b,
            tc.tile_pool(name="xin", bufs=5) as pool_x,
            tc.tile_pool(name="wk1", bufs=2) as pool_1,
            tc.tile_pool(name="wks", bufs=2) as pool_s,
            tc.tile_pool(name="wk2", bufs=2) as pool_2,
            tc.tile_pool(name="outp", bufs=3) as pool_o,
        ):
            # activation() needs SBUF [P,1] bias vectors for non-Copy funcs
            b1c = pool_b.tile([P, 1], F32, tag="b1")
            nc.vector.memset(b1c[:], b1)
            gbias = {}
            for k in nzk:
                s_k = abs(mlp_w[k] / wa)
                gbias[k] = pool_b.tile([P, 1], F32, tag=f"gb{k}", name=f"gb{k}")
                nc.vector.memset(gbias[k][:], s_k * b2)
            mbc = pool_b.tile([P, 1], F32, tag="mb")
            nc.vector.memset(mbc[:], mlp_b)

            for it in range(NTILES):
                n0 = it * P
                xt = pool_x.tile([P, NROWS * MSG], F32, tag="x")
                nc.gpsimd.dma_start(
                    out=xt[:],
                    in_=x[n0 : n0 + P, R0 : R0 + NROWS, :].rearrange(
                        "p r m -> p (r m)"
                    ),
                )

                # conv1: u1 = x_pv + r1*x_ot (STT, 1x either way, so it does
                # the fp32->fp16 narrowing for free), relu-affine in place
                u1 = pool_1.tile([P, 8 * MSG], F16, tag="u1")
                if p1 == 0.0:
                    nc.vector.memset(u1[:], max(b1, 0.0))
                else:
                    nc.vector.scalar_tensor_tensor(
                        out=u1[:],
                        in0=xt[:, ot1 * MSG : ot1 * MSG + 8 * MSG],
                        scalar=r1,
                        in1=xt[:, pv1 * MSG : pv1 * MSG + 8 * MSG],
                        op0=OP.mult,
                        op1=OP.add,
                    )
                    nc.scalar.activation(
                        out=u1[:], in_=u1[:], func=AF.Relu, bias=b1c[:], scale=p1
                    )

                # conv2 in fp16: scale at 4x (tensor_scalar) + add at 2x
                # (tensor_tensor) beats one 1x STT.  NOTE: the TT must NOT
                # write in place over ys -- out aliasing in0 drops the DVE
                # to 1x mode (measured 3.9us vs 2.55us).
                u2 = pool_2.tile([P, 6 * MSG], F16, tag="u2")
                if p2 == 0.0:
                    nc.vector.memset(u2[:], 0.0)
                    u2_scale = 0.0
                else:
                    ys = pool_s.tile([P, 6 * MSG], F16, tag="ys")
                    nc.vector.tensor_scalar_mul(
                        out=ys[:],
                        in0=u1[:, ot2 * MSG : ot2 * MSG + 6 * MSG],
                        scalar1=r2,
                    )
                    nc.vector.tensor_tensor(
                        out=u2[:],
                        in0=ys[:],
                        in1=u1[:, pv2 * MSG : pv2 * MSG + 6 * MSG],
                        op=OP.add,
                    )
                    u2_scale = p2

                # G[k] = s_k*relu(conv2[k]) fused into one ACT op per row,
                # in place on u2's rows
                def row(k):
                    return u2[:, k * MSG : (k + 1) * MSG]

                terms = []  # (tau, row_idx)
                for k in nzk:
                    s_k = abs(mlp_w[k] / wa)
                    nc.scalar.activation(
                        out=row(k),
                        in_=row(k),
                        func=AF.Relu,
                        bias=gbias[k][:],
                        scale=s_k * u2_scale,
                    )
                    terms.append((1 if mlp_w[k] / wa > 0 else -1, k))

                # scalar-free signed pairwise tree, in place into the left
                # operand's row.  TT gets the 2x fp16 mode (vs 1x-only STT).
                # Exactly ONE level-1 pair goes to the GPSIMD engine: a 2nd
                # Pool op measurably slows DVE's 2-port-mode ops (GPSIMD
                # shares SBUF ports with VectorE -- TS went 1228->1789ns).
                pool_terms = []
                while len(terms) >= 4 and len(pool_terms) < 1:
                    (ta, ka), (tb, kb) = terms[-2], terms[-1]
                    terms = terms[:-2]
                    nc.gpsimd.tensor_tensor(
                        out=row(ka),
                        in0=row(ka),
                        in1=row(kb),
                        op=OP.add if ta == tb else OP.subtract,
                    )
                    pool_terms.append((ta, ka))
                while len(terms) > 1:
                    nxt = []
                    for i in range(0, len(terms) - 1, 2):
                        ta, ka = terms[i]
                        tb, kb = terms[i + 1]
                        nc.vector.tensor_tensor(
                            out=row(ka),
                            in0=row(ka),
                            in1=row(kb),
                            op=OP.add if ta == tb else OP.subtract,
                        )
                        nxt.append((ta, ka))
                    if len(terms) % 2:
                        nxt.append(terms[-1])
                    terms = nxt
                for tb, kb in pool_terms:
                    if not terms:
                        terms = [(tb, kb)]
                        continue
                    ta, ka = terms[0]
                    nc.vector.tensor_tensor(
                        out=row(ka),
                        in0=row(ka),
                        in1=row(kb),
                        op=OP.add if ta == tb else OP.subtract,
                    )
                    terms = [(ta, ka)]

                ot = pool_o.tile([P, MSG], F32, tag="o")
                if terms:
                    tau, ka = terms[0]
                    nc.scalar.activation(
                        out=ot[:],
                        in_=row(ka),
                        func=AF.Relu,
                        bias=mbc[:],
                        scale=wa * tau,
                    )
                else:
                    nc.vector.memset(ot[:], max(mlp_b, 0.0))
                nc.sync.dma_start(out=out[n0 : n0 + P, :], in_=ot[:])
    _split_multi_waits(nc)
    return nc


def run(inputs, trace=False, **spmd_kwargs):
    """Build + run on 8 cores. Returns (full_output, BassKernelResults)."""
    msgs = np.asarray(inputs["messages"], dtype=np.float32)
    assert msgs.shape == (N_FULL, L, MSG), msgs.shape
    if not msgs.flags["C_CONTIGUOUS"]:
        msgs = np.ascontiguousarray(msgs)

    c1w = np.asarray(inputs["conv1_w"], dtype=np.float64)
    c2w = np.asarray(inputs["conv2_w"], dtype=np.float64)
    mlw = np.asarray(inputs["mlp_w"], dtype=np.float64)
    nc = build_program(
        float(c1w[0]),
        float(c1w[1]),
        float(np.asarray(inputs["conv1_b"], dtype=np.float64)),
        float(c2w[0]),
        float(c2w[1]),
        float(np.asarray(inputs["conv2_b"], dtype=np.float64)),
        [float(v) for v in mlw],
        float(np.asarray(inputs["mlp_b"], dtype=np.float64)),
    )

    in_maps = [
        {"x": msgs[i * N_LOCAL : (i + 1) * N_LOCAL]} for i in range(N_CORES)
    ]
    res = run_bass_kernel_spmd(
        nc, in_maps, core_ids=list(range(N_CORES)), trace=trace, **spmd_kwargs
    )
    full = np.concatenate([r["out"] for r in res.results], axis=0)
    return full, res


def kernel(**inputs) -> np.ndarray:
    return run(inputs, trace=False)[0]
